# revision 1
# baseline (speedup 1.0000x reference)
import sys

import numpy as np

try:
    import concourse.bass as bass
except ImportError:
    sys.path.insert(0, "/opt/trn_rl_repo")
    import concourse.bass as bass

import concourse.bacc as bacc
import concourse.mybir as mybir
import concourse.tile as tile
from concourse.bass_utils import run_bass_kernel_spmd

F32 = mybir.dt.float32
B, S, D = 2, 2048, 1024
NH, DH = 16, 64
HPC = 4            # heads per core
HF = HPC * DH      # 256 per-core head features
TQ = S // 4        # 512: t-chunk / i-chunk / per-core output quarter
NJT = S // 128     # 16 j-tiles of 128
SCALE = 1.0 / float(np.sqrt(DH))

_CACHE = {}


def _build_graph(variant="full", reps=1):
    noag = variant in ("noag", "noattn", "noag2")
    noattn = variant == "noattn"
    smallexp = variant == "smallexp"
    noxdma = variant == "noxdma"
    nc = bacc.Bacc(num_devices=8)

    xqT = nc.dram_tensor("xqT", [D, S], F32, kind="ExternalInput")
    xkT = nc.dram_tensor("xkT", [D, S], F32, kind="ExternalInput")
    xvT = nc.dram_tensor("xvT", [D, S], F32, kind="ExternalInput")
    wqT = nc.dram_tensor("wqT", [D, HF], F32, kind="ExternalInput")
    wkT = nc.dram_tensor("wkT", [D, HF], F32, kind="ExternalInput")
    wvT = nc.dram_tensor("wvT", [D, HF], F32, kind="ExternalInput")
    woT = nc.dram_tensor("woT", [D, D], F32, kind="ExternalInput")
    dmask = nc.dram_tensor("dmask", [128, 128], F32, kind="ExternalInput")
    # sel[0, j] == 1 iff this core's output quarter is j (used to predicate
    # which AllGather result feeds the output projection)
    sel = nc.dram_tensor("sel", [1, 4], mybir.dt.uint32, kind="ExternalInput")
    out_q = nc.dram_tensor("out_q", [TQ, D], F32, kind="ExternalOutput")

    Exp = mybir.ActivationFunctionType.Exp

    with tile.TileContext(nc) as tc:
        sel_regs = []
        if not noag:
            for j in range(4):
                r = nc.sync.alloc_register(f"selreg{j}")
                nc.sync.reg_load(r, sel[0:1, j:j + 1])
                sel_regs.append(nc.sync.snap(r, donate=True, min_val=0, max_val=1))
        with (
            tc.tile_pool(name="dram", bufs=1, space="DRAM") as dramp,
            tc.tile_pool(name="const", bufs=1) as constp,
            tc.tile_pool(name="persist", bufs=1) as pers,
            tc.tile_pool(name="weights", bufs=1) as wpool,
            tc.tile_pool(name="xstage", bufs=3) as xpool,
            tc.tile_pool(name="attn", bufs=(3 if variant == "full" else 6 if variant == "opt3" else 4)) as apool,
            tc.tile_pool(name="ctx", bufs=2) as cpool,
            tc.tile_pool(name="rb", bufs=2) as rbpool,
            tc.tile_pool(name="rv", bufs=2) as rvpool,
            tc.tile_pool(name="obuf", bufs=(2 if variant == "opt3" else 3)) as obp,
            tc.tile_pool(name="ps_mm", bufs=(1 if variant == "opt2" else 2), space="PSUM") as ps_mm,
            tc.tile_pool(name="ps_s", bufs=(2 if variant == "full" else 4 if variant == "opt2" else 3), space="PSUM") as ps_s,
            tc.tile_pool(name="ps_ctx", bufs=2, space="PSUM") as ps_ctx,
            tc.tile_pool(name="ps_b", bufs=1, space="PSUM") as ps_b,
        ):
            ccin = [dramp.tile([HF, TQ], F32, name=f"ccin{j}") for j in range(4)]
            agout = [
                dramp.tile([4 * HF, TQ], F32, name=f"agout{j}") for j in range(4)
            ]

            dmask_sb = constp.tile([128, 128], F32, name="dmask_sb")
            nc.sync.dma_start(dmask_sb[:], dmask[:, :])
            ones_sb = constp.tile([1, DH], F32, name="ones_sb")
            nc.vector.memset(ones_sb[:], 1.0)

            wq_sb = wpool.tile([128, 8, HF], F32, name="wq_sb")
            wk_sb = wpool.tile([128, 8, HF], F32, name="wk_sb")
            wv_sb = wpool.tile([128, 8, HF], F32, name="wv_sb")
            nc.sync.dma_start(wq_sb[:], wqT[:, :].rearrange("(n p) o -> p n o", p=128))
            nc.sync.dma_start(wk_sb[:], wkT[:, :].rearrange("(n p) o -> p n o", p=128))
            nc.sync.dma_start(wv_sb[:], wvT[:, :].rearrange("(n p) o -> p n o", p=128))
            wo_sb = wpool.tile([128, 8, D], F32, name="wo_sb")
            nc.sync.dma_start(wo_sb[:], woT[:, :].rearrange("(n p) d -> p n d", p=128))

            # Persistent Q^T/K^T (2 tiles each: heads (0,1) and (2,3) stacked on
            # partitions) and V in natural orientation augmented with a ones
            # column (row 64 of the AV product becomes the softmax denominator).
            QT = [pers.tile([128, S], F32, name=f"QT{u}") for u in range(2)]
            KT = [pers.tile([128, S], F32, name=f"KT{u}") for u in range(2)]
            Vb = pers.tile([128, NJT * HPC, DH + 1], F32, name="Vb")
            nc.vector.memset(Vb[:, :, DH], 1.0)

            for rep in range(reps):
             for tcc in range(4):
                xq_sb = xpool.tile([128, 8, TQ], F32, name="xst")
                xk_sb = xpool.tile([128, 8, TQ], F32, name="xst")
                xv_sb = xpool.tile([128, 8, TQ], F32, name="xst")
                for xs, src in ((xq_sb, xqT), (xk_sb, xkT), (xv_sb, xvT)):
                    src_ap = src[:, bass.ts(tcc, TQ)].rearrange(
                        "(n p) t -> p n t", p=128
                    )
                    if noxdma:
                        nc.sync.dma_start(xs[:, 0:2, :], src_ap[:, 0:2, :])
                    else:
                        nc.sync.dma_start(xs[:], src_ap)

                # Q/K projections for this t-chunk: psum[o128, t512] over 8 d-tiles
                for xs, w_sb, dst in ((xq_sb, wq_sb, QT), (xk_sb, wk_sb, KT)):
                    for u in range(2):
                        ps = ps_mm.tile([128, TQ], F32, name="ps")
                        for kd in range(8):
                            nc.tensor.matmul(
                                ps[:],
                                w_sb[:, kd, bass.ts(u, 128)],
                                xs[:, kd, :],
                                start=(kd == 0),
                                stop=(kd == 7),
                            )
                        nc.vector.tensor_copy(dst[u][:, bass.ts(tcc, TQ)], ps[:])

                # V projection: natural orientation [t128, feat256] per j-tile
                for jl in range(4):
                    jt = tcc * 4 + jl
                    psv = ps_mm.tile([128, TQ], F32, name="ps")
                    for kd in range(8):
                        nc.tensor.matmul(
                            psv[:, 0:HF],
                            xv_sb[:, kd, bass.ts(jl, 128)],
                            wv_sb[:, kd, :],
                            start=(kd == 0),
                            stop=(kd == 7),
                        )
                    nc.vector.tensor_copy(
                        Vb[:, jt * HPC:(jt + 1) * HPC, 0:DH],
                        psv[:, 0:HF].rearrange("p (h k) -> p h k", k=DH),
                    )

                # Attention for i-chunk ic == tcc (all K/V up to j<=i now exist)
                ic = tcc
                n_jt = 4 * ic + 4
                for h in range(HPC if not noattn else 0):
                    u, po = h // 2, (h % 2) * DH
                    q_ap = QT[u][po:po + DH, bass.ts(ic, TQ)]
                    pctx = ps_ctx.tile([DH + 1, TQ], F32, name="pctx")
                    ats = []
                    for jt in range(n_jt):
                        ps = ps_s.tile([128, TQ], F32, name="ps_sc")
                        nc.tensor.matmul(
                            ps[:],
                            KT[u][po:po + DH, bass.ts(jt, 128)],
                            q_ap,
                            start=True,
                            stop=True,
                        )
                        at = apool.tile([128, TQ], F32, name="at")
                        p = jt - 4 * ic
                        if p > 0:
                            nc.vector.memset(at[:, 0:p * 128], 0.0)
                        if p >= 0:
                            nc.vector.tensor_add(
                                ps[:, bass.ts(p, 128)],
                                ps[:, bass.ts(p, 128)],
                                dmask_sb[:],
                            )
                            hi = p * 128 + 128 if smallexp else TQ
                            nc.scalar.activation(
                                at[:, p * 128:hi], ps[:, p * 128:hi], Exp, scale=SCALE
                            )
                        else:
                            hi = 128 if smallexp else TQ
                            nc.scalar.activation(
                                at[:, 0:hi], ps[:, 0:hi], Exp, scale=SCALE
                            )
                        ats.append(at)
                        # AV accumulation skewed behind scores for PE/ACT
                        # pipelining (deeper ACT runahead on opt3)
                        skew = 2 if variant == "opt3" else 1
                        if jt >= skew:
                            pv = jt - skew
                            nc.tensor.matmul(
                                pctx[:],
                                Vb[:, pv * HPC + h, :],
                                ats[pv][:],
                                start=(pv == 0),
                                stop=False,
                            )
                    for pv in range(max(n_jt - skew, 0), n_jt):
                        nc.tensor.matmul(
                            pctx[:],
                            Vb[:, pv * HPC + h, :],
                            ats[pv][:],
                            start=(pv == 0),
                            stop=(pv == n_jt - 1),
                        )

                    # Normalize: row DH of pctx is the denominator. Broadcast
                    # 1/denom across 64 partitions via a rank-1 matmul.
                    rv = rvpool.tile([1, TQ], F32, name="rvec")
                    nc.vector.reciprocal(rv[:], pctx[DH:DH + 1, :])
                    pb = ps_b.tile([DH, TQ], F32, name="pb")
                    nc.tensor.matmul(pb[:], ones_sb[:], rv[:], start=True, stop=True)
                    rb = rbpool.tile([DH, TQ], F32, name="rbt")
                    nc.vector.tensor_copy(rb[:], pb[:])
                    ctxT = cpool.tile([DH, TQ], F32, name="ctxT")
                    nc.vector.tensor_mul(ctxT[:], pctx[0:DH, :], rb[:])
                    nc.sync.dma_start(
                        ccin[ic][h * DH:(h + 1) * DH, :], ctxT[:]
                    )

                # Gather all 16 heads' ctxT for this i-chunk across the 4-core
                # group (concat by group rank = head-major feature order).
                if not noag:
                    nc.gpsimd.collective_compute(
                        "AllGather",
                        mybir.AluOpType.bypass,
                        replica_groups=[[0, 1, 2, 3], [4, 5, 6, 7]],
                        ins=[ccin[ic].opt()],
                        outs=[agout[ic].opt()],
                    )

             # Stage the gather result for THIS core's output quarter: four
             # predicated DMAs, exactly one of which fires at runtime.
             cst = wpool.tile([128, 8, TQ], F32, name="cst")
             if noag:
                if not noattn:
                    for j in range(4):
                        nc.sync.dma_start(
                            cst[:, 2 * j:2 * j + 2, :],
                            ccin[j][:, :].rearrange("(n p) t -> p n t", p=128),
                        )
             else:
                for j in range(4):
                    nc.sync.dma_start(
                        cst[:],
                        agout[j][:, :].rearrange("(n p) t -> p n t", p=128),
                        cond=sel_regs[j],
                    )
             for tt in range(4):
                for dc in range(2):
                    pso = ps_mm.tile([128, TQ], F32, name="ps")
                    for kt in range(8):
                        nc.tensor.matmul(
                            pso[:],
                            cst[:, kt, bass.ts(tt, 128)],
                            wo_sb[:, kt, bass.ts(dc, TQ)],
                            start=(kt == 0),
                            stop=(kt == 7),
                        )
                    ob = obp.tile([128, TQ], F32, name="ob")
                    nc.scalar.copy(ob[:], pso[:])
                    nc.sync.dma_start(
                        out_q[bass.ts(tt, 128), bass.ts(dc, TQ)], ob[:]
                    )

    nc.finalize()
    return nc


def _make_in_maps(inputs):
    query, key, value = inputs["query"], inputs["key"], inputs["value"]
    mask = inputs["mask"]
    Wq, Wk, Wv, Wo = inputs["Wq"], inputs["Wk"], inputs["Wv"], inputs["Wo"]

    dmask_blk = np.where(
        np.asarray(mask[:128, :128]).T, np.float32(0.0), np.float32(-1e9)
    ).astype(np.float32)
    woT_full = np.ascontiguousarray(np.asarray(Wo, np.float32).T)

    in_maps = []
    for c in range(8):
        b, r = divmod(c, 4)
        rs = slice(r * HF, (r + 1) * HF)
        in_maps.append(
            {
                "xqT": np.ascontiguousarray(np.asarray(query[b], np.float32).T),
                "xkT": np.ascontiguousarray(np.asarray(key[b], np.float32).T),
                "xvT": np.ascontiguousarray(np.asarray(value[b], np.float32).T),
                "wqT": np.ascontiguousarray(np.asarray(Wq[rs], np.float32).T),
                "wkT": np.ascontiguousarray(np.asarray(Wk[rs], np.float32).T),
                "wvT": np.ascontiguousarray(np.asarray(Wv[rs], np.float32).T),
                "woT": woT_full,
                "dmask": dmask_blk,
                "sel": (np.arange(4, dtype=np.uint32) == r).astype(np.uint32)[None, :],
            }
        )
    return in_maps


def _run(inputs, trace=False):
    if "nc" not in _CACHE:
        _CACHE["nc"] = _build_graph()
    nc = _CACHE["nc"]
    in_maps = _make_in_maps(inputs)
    res = run_bass_kernel_spmd(nc, in_maps, core_ids=list(range(8)), trace=trace)

    out = np.empty((B, S, D), np.float32)
    for c in range(8):
        b, q = divmod(c, 4)
        out[b, q * TQ:(q + 1) * TQ, :] = np.asarray(res.results[c]["out_q"])
    return out, res


def kernel(**inputs):
    out, _ = _run(inputs, trace=False)
    return out



# revision 2
# speedup vs baseline: 7.2382x; 7.2382x over previous
import sys

import numpy as np

try:
    import concourse.bass as bass
except ImportError:
    sys.path.insert(0, "/opt/trn_rl_repo")
    import concourse.bass as bass

import concourse.bacc as bacc
import concourse.mybir as mybir
import concourse.tile as tile
from concourse.bass_utils import run_bass_kernel_spmd

import ml_dtypes

F32 = mybir.dt.float32
BF = mybir.dt.bfloat16
BF_NP = ml_dtypes.bfloat16
B, S, D = 2, 2048, 1024
NH, DH = 16, 64
HPC = 4            # heads per core
HF = HPC * DH      # 256 per-core head features
TQ = S // 4        # 512: t-chunk / i-chunk / per-core output quarter
NJT = S // 128     # 16 j-tiles of 128
SCALE = 1.0 / float(np.sqrt(DH))
SKEW = 2           # AV lags scores by SKEW j-tiles (ACT runahead)

_CACHE = {}


def _build_graph(single=False, variant="", reps=1):
    # single=True: one-core diagnostic build for TimelineSim — no collectives,
    # no predicated DMAs; numerics of the output projection are garbage but
    # the instruction mix/timing matches the real kernel.
    # variant flags (comma-separated): "mmbcast" = rank-1-matmul denominator
    # broadcast instead of gpsimd; "headexp" = per-head 2D exp instead of
    # pair-batched 3D; "noil" = no proj interleave into attention;
    # "norestrict" = full-width scores/exp/AV (no causal column restriction).
    flags = set(variant.split(",")) if variant else set()
    mmbcast = "mmbcast" in flags
    headexp = "headexp" in flags
    noil = "noil" in flags
    norestrict = "norestrict" in flags
    projonly = "projonly" in flags   # skip attention + output projection
    noout = "noout" in flags         # skip AllGather + output projection
    if projonly:
        noil = True
    nc = bacc.Bacc(num_devices=1 if single else 8)

    xqT = nc.dram_tensor("xqT", [D, S], BF, kind="ExternalInput")
    xkT = nc.dram_tensor("xkT", [D, S], BF, kind="ExternalInput")
    xvT = nc.dram_tensor("xvT", [D, S], BF, kind="ExternalInput")
    wqT = nc.dram_tensor("wqT", [D, HF], BF, kind="ExternalInput")
    wkT = nc.dram_tensor("wkT", [D, HF], BF, kind="ExternalInput")
    wvT = nc.dram_tensor("wvT", [D, HF], BF, kind="ExternalInput")
    woT = nc.dram_tensor("woT", [D, D], BF, kind="ExternalInput")
    dmask = nc.dram_tensor("dmask", [128, 128], F32, kind="ExternalInput")
    # sel[0, j] == 1 iff this core's output quarter is j (predicates which
    # AllGather result feeds the output projection)
    sel = nc.dram_tensor("sel", [1, 4], mybir.dt.uint32, kind="ExternalInput")
    out_q = nc.dram_tensor("out_q", [TQ, D], F32, kind="ExternalOutput")

    Exp = mybir.ActivationFunctionType.Exp

    with tile.TileContext(nc) as tc:
        sel_regs = []
        if not single:
            for j in range(4):
                r = nc.sync.alloc_register(f"selreg{j}")
                nc.sync.reg_load(r, sel[0:1, j:j + 1])
                sel_regs.append(nc.sync.snap(r, donate=True, min_val=0, max_val=1))
        with (
            tc.tile_pool(name="dram", bufs=1, space="DRAM") as dramp,
            tc.tile_pool(name="const", bufs=1) as constp,
            tc.tile_pool(name="persist", bufs=1) as pers,
            tc.tile_pool(name="weights", bufs=1) as wpool,
            tc.tile_pool(name="xstage", bufs=6) as xpool,
            tc.tile_pool(name="attn", bufs=6) as apool,
            tc.tile_pool(name="ctx", bufs=4) as cpool,
            tc.tile_pool(name="rv", bufs=4) as rvpool,
            tc.tile_pool(name="rb", bufs=4) as rbpool,
            tc.tile_pool(name="obuf", bufs=3) as obp,
            tc.tile_pool(name="ps_mm", bufs=2, space="PSUM") as ps_mm,
            tc.tile_pool(name="ps_s", bufs=(4 if headexp else 2), space="PSUM") as ps_s,
            tc.tile_pool(name="ps_ctx", bufs=2, space="PSUM") as ps_ctx,
        ):
            ccin = [dramp.tile([HF, TQ], BF, name=f"ccin{j}") for j in range(4)]
            agout = [
                dramp.tile([4 * HF, TQ], BF, name=f"agout{j}") for j in range(4)
            ]

            dmask_sb = constp.tile([128, 128], F32, name="dmask_sb")
            nc.sync.dma_start(dmask_sb[:], dmask[:, :])
            ones_sb = None
            if mmbcast:
                ones_sb = constp.tile([1, DH], F32, name="ones_sb")
                nc.vector.memset(ones_sb[:], 1.0)

            wq_sb = wpool.tile([128, 8, HF], BF, name="wq_sb")
            wk_sb = wpool.tile([128, 8, HF], BF, name="wk_sb")
            wv_sb = wpool.tile([128, 8, HF], BF, name="wv_sb")
            nc.sync.dma_start(wq_sb[:], wqT[:, :].rearrange("(n p) o -> p n o", p=128))
            nc.sync.dma_start(wk_sb[:], wkT[:, :].rearrange("(n p) o -> p n o", p=128))
            nc.sync.dma_start(wv_sb[:], wvT[:, :].rearrange("(n p) o -> p n o", p=128))
            wo_sb = wpool.tile([128, 8, D], BF, name="wo_sb")
            nc.sync.dma_start(wo_sb[:], woT[:, :].rearrange("(n p) d -> p n d", p=128))

            # Persistent Q^T/K^T (2 tiles each: heads (0,1) and (2,3) stacked on
            # partitions at offsets 0/64 — the offset doubles as the PE row
            # group, so the two heads' score matmuls run concurrently) and V in
            # natural orientation augmented with a ones column (row 64 of the
            # AV product becomes the softmax denominator).
            QT = [pers.tile([128, S], BF, name=f"QT{u}") for u in range(2)]
            KT = [pers.tile([128, S], BF, name=f"KT{u}") for u in range(2)]
            Vb = pers.tile([128, NJT * HPC, DH + 1], BF, name="Vb")
            nc.vector.memset(Vb[:, :, DH], 1.0)

            def load_x(tcc):
                tiles = []
                for src in (xqT, xkT, xvT):
                    xs = xpool.tile([128, 8, TQ], BF, name="xst")
                    nc.sync.dma_start(
                        xs[:],
                        src[:, bass.ts(tcc, TQ)].rearrange("(n p) t -> p n t", p=128),
                    )
                    tiles.append(xs)
                return tiles

            def emit_group(tcc, xt, g):
                """One projection PSUM-group for chunk tcc. g 0-3: Q/K
                projections (xs, u); g 4-7: V projection j-subtile."""
                xq_sb, xk_sb, xv_sb = xt
                if g < 4:
                    xs, w_sb, dst = (
                        (xq_sb, wq_sb, QT) if g < 2 else (xk_sb, wk_sb, KT)
                    )
                    u = g % 2
                    ps = ps_mm.tile([128, TQ], F32, name="ps")
                    for kd in range(8):
                        nc.tensor.matmul(
                            ps[:],
                            w_sb[:, kd, bass.ts(u, 128)],
                            xs[:, kd, :],
                            start=(kd == 0),
                            stop=(kd == 7),
                        )
                    nc.vector.tensor_copy(dst[u][:, bass.ts(tcc, TQ)], ps[:])
                else:
                    jl = g - 4
                    jt = tcc * 4 + jl
                    psv = ps_mm.tile([128, TQ], F32, name="ps")
                    for kd in range(8):
                        nc.tensor.matmul(
                            psv[:, 0:HF],
                            xv_sb[:, kd, bass.ts(jl, 128)],
                            wv_sb[:, kd, :],
                            start=(kd == 0),
                            stop=(kd == 7),
                        )
                    nc.vector.tensor_copy(
                        Vb[:, jt * HPC:(jt + 1) * HPC, 0:DH],
                        psv[:, 0:HF].rearrange("p (h k) -> p h k", k=DH),
                    )

            for rep in range(reps):
             cur_x = load_x(0)
             for g in range(8):
                emit_group(0, cur_x, g)

             for tcc in range(4):
                ic = tcc
                n_jt = 4 * ic + 4
                nxt_x = load_x(tcc + 1) if tcc < 3 else None
                pending = (
                    [(tcc + 1, nxt_x, g) for g in range(8)] if tcc < 3 else []
                )
                nslots = 2 * n_jt
                emitted = 0
                slot = 0

                for u in range(2 if not projonly else 0):
                    pctx = [
                        ps_ctx.tile([DH + 1, TQ], F32, name="pctx")
                        for _ in range(2)
                    ]
                    ats = []

                    def emit_av(pv, u=u, pctx=pctx, ats=ats, n_jt=n_jt):
                        pat, plo = ats[pv]
                        av_lo = 0 if norestrict else plo
                        for hh in range(2):
                            nc.tensor.matmul(
                                pctx[hh][:, av_lo:TQ],
                                Vb[:, pv * HPC + 2 * u + hh, :],
                                pat[hh][:, av_lo:TQ] if headexp
                                else pat[0][:, hh, av_lo:TQ],
                                start=(pv == 0),
                                stop=(pv == n_jt - 1),
                            )

                    for jt in range(n_jt):
                        p = jt - 4 * ic
                        lo = max(p, 0) * 128
                        sc_lo = 0 if norestrict else lo
                        if headexp:
                            ps2 = [
                                ps_s.tile([128, TQ], F32, name="ps_sc")
                                for _ in range(2)
                            ]
                        else:
                            ps = ps_s.tile([128, 2, TQ], F32, name="ps_sc")
                            ps2 = [ps[:, 0], ps[:, 1]]
                        for hh in range(2):
                            po = hh * DH
                            nc.tensor.matmul(
                                ps2[hh][:, sc_lo:TQ],
                                KT[u][po:po + DH, bass.ts(jt, 128)],
                                QT[u][po:po + DH, ic * TQ + sc_lo:(ic + 1) * TQ],
                                start=True,
                                stop=True,
                            )
                        if p >= 0:
                            for hh in range(2):
                                nc.vector.tensor_add(
                                    ps2[hh][:, p * 128:p * 128 + 128],
                                    ps2[hh][:, p * 128:p * 128 + 128],
                                    dmask_sb[:],
                                )
                        if headexp:
                            at = [
                                apool.tile([128, TQ], BF, name="at")
                                for _ in range(2)
                            ]
                            for hh in range(2):
                                if norestrict and p > 0:
                                    nc.vector.memset(at[hh][:, 0:lo], 0.0)
                                nc.scalar.activation(
                                    at[hh][:, lo:TQ], ps2[hh][:, lo:TQ],
                                    Exp, scale=SCALE,
                                )
                        else:
                            atp = apool.tile([128, 2, TQ], BF, name="at")
                            if norestrict and p > 0:
                                nc.vector.memset(atp[:, :, 0:lo], 0.0)
                            nc.scalar.activation(
                                atp[:, :, lo:TQ], ps[:, :, lo:TQ],
                                Exp, scale=SCALE,
                            )
                            at = [atp]
                        ats.append((at, lo))
                        if jt >= SKEW:
                            emit_av(jt - SKEW)
                        slot += 1
                        while (
                            not noil
                            and emitted < len(pending)
                            and emitted * nslots // 8 < slot
                        ):
                            emit_group(*pending[emitted])
                            emitted += 1
                    for pv in range(max(n_jt - SKEW, 0), n_jt):
                        emit_av(pv)

                    # Normalize: row DH of pctx is the denominator; broadcast
                    # its reciprocal across the 64 feature partitions.
                    for hh in range(2):
                        h = 2 * u + hh
                        rv = rvpool.tile([1, TQ], F32, name="rvec")
                        nc.vector.reciprocal(rv[:], pctx[hh][DH:DH + 1, :])
                        rvb = rbpool.tile([DH, TQ], F32, name="rvb")
                        if mmbcast:
                            pb = ps_mm.tile([128, TQ], F32, name="ps")
                            nc.tensor.matmul(
                                pb[0:DH, :],
                                ones_sb[:],
                                rv[:],
                                start=True,
                                stop=True,
                            )
                            nc.vector.tensor_copy(rvb[:], pb[0:DH, :])
                        else:
                            nc.gpsimd.partition_broadcast(rvb[:], rv[:])
                        ctxT = cpool.tile([DH, TQ], BF, name="ctxT")
                        nc.vector.tensor_mul(ctxT[:], pctx[hh][0:DH, :], rvb[:])
                        nc.sync.dma_start(
                            ccin[ic][h * DH:(h + 1) * DH, :], ctxT[:]
                        )

                while emitted < len(pending):
                    emit_group(*pending[emitted])
                    emitted += 1

                # Gather all 16 heads' ctxT for this i-chunk across the 4-core
                # group (concat by group rank = head-major feature order).
                if not (single or projonly or noout):
                    nc.gpsimd.collective_compute(
                        "AllGather",
                        mybir.AluOpType.bypass,
                        replica_groups=[[0, 1, 2, 3], [4, 5, 6, 7]],
                        ins=[ccin[ic].opt()],
                        outs=[agout[ic].opt()],
                    )

             if projonly or noout:
                dummy = obp.tile([128, TQ], F32, name="ob")
                nc.vector.memset(dummy[:], 0.0)
                for tt in range(4):
                    for dc in range(2):
                        nc.sync.dma_start(
                            out_q[bass.ts(tt, 128), bass.ts(dc, TQ)], dummy[:]
                        )
             else:
                # Stage the gather result for THIS core's output quarter: four
                # predicated DMAs, exactly one of which fires at runtime.
                cst = wpool.tile([128, 8, TQ], BF, name="cst")
                if single:
                    for j in range(4):
                        nc.sync.dma_start(
                            cst[:, 2 * j:2 * j + 2, :],
                            ccin[j][:, :].rearrange("(n p) t -> p n t", p=128),
                        )
                else:
                    for j in range(4):
                        nc.sync.dma_start(
                            cst[:],
                            agout[j][:, :].rearrange("(n p) t -> p n t", p=128),
                            cond=sel_regs[j],
                        )
                for tt in range(4):
                    for dc in range(2):
                        pso = ps_mm.tile([128, TQ], F32, name="ps")
                        for kt in range(8):
                            nc.tensor.matmul(
                                pso[:],
                                cst[:, kt, bass.ts(tt, 128)],
                                wo_sb[:, kt, bass.ts(dc, TQ)],
                                start=(kt == 0),
                                stop=(kt == 7),
                            )
                        ob = obp.tile([128, TQ], F32, name="ob")
                        nc.vector.tensor_copy(ob[:], pso[:])
                        nc.sync.dma_start(
                            out_q[bass.ts(tt, 128), bass.ts(dc, TQ)], ob[:]
                        )

    nc.finalize()
    return nc


def _make_in_maps(inputs):
    query, key, value = inputs["query"], inputs["key"], inputs["value"]
    mask = inputs["mask"]
    Wq, Wk, Wv, Wo = inputs["Wq"], inputs["Wk"], inputs["Wv"], inputs["Wo"]

    dmask_blk = np.where(
        np.asarray(mask[:128, :128]).T, np.float32(0.0), np.float32(-1e9)
    ).astype(np.float32)
    woT_full = np.ascontiguousarray(np.asarray(Wo, np.float32).T).astype(BF_NP)
    xT = [
        np.ascontiguousarray(np.asarray(t, np.float32).T).astype(BF_NP)
        for t in (query[0], query[1], key[0], key[1], value[0], value[1])
    ]

    in_maps = []
    for c in range(8):
        b, r = divmod(c, 4)
        rs = slice(r * HF, (r + 1) * HF)
        in_maps.append(
            {
                "xqT": xT[b],
                "xkT": xT[2 + b],
                "xvT": xT[4 + b],
                "wqT": np.ascontiguousarray(
                    np.asarray(Wq[rs], np.float32).T
                ).astype(BF_NP),
                "wkT": np.ascontiguousarray(
                    np.asarray(Wk[rs], np.float32).T
                ).astype(BF_NP),
                "wvT": np.ascontiguousarray(
                    np.asarray(Wv[rs], np.float32).T
                ).astype(BF_NP),
                "woT": woT_full,
                "dmask": dmask_blk,
                "sel": (np.arange(4, dtype=np.uint32) == r).astype(np.uint32)[None, :],
            }
        )
    return in_maps


def _run(inputs, trace=False, variant=""):
    key = "nc" + variant
    if key not in _CACHE:
        _CACHE[key] = _build_graph(variant=variant)
    nc = _CACHE[key]
    in_maps = _make_in_maps(inputs)
    res = run_bass_kernel_spmd(nc, in_maps, core_ids=list(range(8)), trace=trace)

    out = np.empty((B, S, D), np.float32)
    for c in range(8):
        b, q = divmod(c, 4)
        out[b, q * TQ:(q + 1) * TQ, :] = np.asarray(res.results[c]["out_q"])
    return out, res


def kernel(**inputs):
    out, _ = _run(inputs, trace=False)
    return out
